# revision 1
# baseline (speedup 1.0000x reference)
"""Two-layer LSTM (B=256, T=256, D=128, H=1024, O=128) on 8 trn2 NeuronCores.

Strategy: 8-way tensor-parallel over the 4H gate dimension.  Core j owns the
j-th 128-row slice of each gate block (i/f/g/o) of every weight matrix, i.e.
512 gate rows per layer, so it computes exactly the gates needed to update its
own 128-wide slice of h and c.  State is kept transposed (h^T [H, B]) so the
per-step matmuls are
    g^T[512, B] = W_chunk[512, K] @ h^T[K, B]
with the weight tiles stationary (lhsT) and h^T the moving operand (N=256).
Cell state c stays core-local in fp32; h chunks are all-gathered (fp16,
128KB/rank) once per superstep.  The two layers are pipelined: superstep s
computes layer0 step s and layer1 step s-1, so layer1's matmuls hide the
all-gather latency of layer0's recurrence.  A final AllGather + on-device
projection produces out^T = W_out @ h1_T; the host transposes it back.
"""

import numpy as np

import concourse.bass as bass
from concourse import bacc
import concourse.mybir as mybir
import concourse.tile as tile
from concourse.bass_utils import run_bass_kernel_spmd

B, T, D, H, O = 256, 256, 128, 1024, 128
NC = 8
HC = H // NC          # 128 h rows per core
GC = 4 * HC           # 512 gate rows per core
KH = H // 128         # 8 k-chunks over H
FP = mybir.dt.float16
F32 = mybir.dt.float32
AFT = mybir.ActivationFunctionType
RG = [list(range(NC))]


def _act_block(nc, apool, g, bb, c_sb, tag):
    """LSTM gate activations + cell update for one 128-slice.

    g: psum tiles [i, f, g, o] each [128, B] fp32.  bb: bias tile [128, 4].
    c_sb: fp32 cell tile [128, B], updated in place (full overwrite).
    Returns new h chunk [128, B] fp16."""
    it = apool.tile([HC, B], F32, tag=tag + "i")
    ft = apool.tile([HC, B], F32, tag=tag + "f")
    gt = apool.tile([HC, B], F32, tag=tag + "g")
    ot = apool.tile([HC, B], F32, tag=tag + "o")
    nc.scalar.activation(it[:], g[0][:], AFT.Sigmoid)
    nc.scalar.activation(ft[:], g[1][:], AFT.Sigmoid)
    nc.scalar.activation(gt[:], g[2][:], AFT.Tanh)
    nc.scalar.activation(ot[:], g[3][:], AFT.Sigmoid)
    fc = apool.tile([HC, B], F32, tag=tag + "fc")
    ig = apool.tile([HC, B], F32, tag=tag + "ig")
    nc.vector.tensor_mul(fc[:], ft[:], c_sb[:])
    nc.vector.tensor_mul(ig[:], it[:], gt[:])
    nc.vector.tensor_add(c_sb[:], fc[:], ig[:])
    tc_ = apool.tile([HC, B], F32, tag=tag + "tc")
    nc.scalar.activation(tc_[:], c_sb[:], AFT.Tanh)
    hnew = apool.tile([HC, B], FP, tag=tag + "h")
    nc.vector.tensor_mul(hnew[:], ot[:], tc_[:])
    return hnew


def build(t_steps):
    nc = bass.Bass(num_devices=NC)

    xT = nc.dram_tensor("xT", [t_steps, D, B], FP, kind="ExternalInput")
    w0i = nc.dram_tensor("w0i", [D, GC], FP, kind="ExternalInput")
    w0h = nc.dram_tensor("w0h", [KH, 128, GC], FP, kind="ExternalInput")
    w1i = nc.dram_tensor("w1i", [KH, 128, GC], FP, kind="ExternalInput")
    w1h = nc.dram_tensor("w1h", [KH, 128, GC], FP, kind="ExternalInput")
    wo = nc.dram_tensor("wo", [KH, 128, O], FP, kind="ExternalInput")
    b0 = nc.dram_tensor("b0", [1, GC], FP, kind="ExternalInput")
    b1 = nc.dram_tensor("b1", [1, GC], FP, kind="ExternalInput")
    bo = nc.dram_tensor("bo", [1, O], FP, kind="ExternalInput")
    ones = nc.dram_tensor("ones", [1, B], FP, kind="ExternalInput")
    h0T = nc.dram_tensor("h0T", [KH, 128, B], FP, kind="ExternalInput")
    hci = nc.dram_tensor("hci", [HC, B], FP, kind="ExternalInput")
    c0T = nc.dram_tensor("c0T", [HC, B], F32, kind="ExternalInput")
    outT = nc.dram_tensor("outT", [O, B], F32, kind="ExternalOutput")

    with tile.TileContext(nc) as tc:
        with (
            tc.tile_pool(name="wpool", bufs=1) as wpool,
            tc.tile_pool(name="spool", bufs=1) as spool,
            tc.tile_pool(name="xpool", bufs=3) as xpool,
            tc.tile_pool(name="apool", bufs=2) as apool,
            tc.tile_pool(name="ppool", bufs=1, space="PSUM") as ppool,
            tc.tile_pool(name="dpool", bufs=2, space="DRAM") as dpool,
        ):
            w0i_sb = wpool.tile([D, GC], FP, tag="w0i")
            w0h_sb = wpool.tile([128, KH * GC], FP, tag="w0h")
            w1i_sb = wpool.tile([128, KH * GC], FP, tag="w1i")
            w1h_sb = wpool.tile([128, KH * GC], FP, tag="w1h")
            wo_sb = wpool.tile([128, KH * O], FP, tag="wo")
            b0_sb = wpool.tile([1, GC], FP, tag="b0")
            b1_sb = wpool.tile([1, GC], FP, tag="b1")
            bo_sb = wpool.tile([1, O], FP, tag="bo")
            ones_sb = wpool.tile([1, B], FP, tag="ones")
            nc.sync.dma_start(w0i_sb[:], w0i[:])
            nc.sync.dma_start(w0h_sb[:].rearrange("p (k m) -> p k m", k=KH), w0h[:].rearrange("k p m -> p k m"))
            nc.sync.dma_start(w1i_sb[:].rearrange("p (k m) -> p k m", k=KH), w1i[:].rearrange("k p m -> p k m"))
            nc.sync.dma_start(w1h_sb[:].rearrange("p (k m) -> p k m", k=KH), w1h[:].rearrange("k p m -> p k m"))
            nc.sync.dma_start(wo_sb[:].rearrange("p (k m) -> p k m", k=KH), wo[:].rearrange("k p m -> p k m"))
            nc.sync.dma_start(b0_sb[:], b0[:])
            nc.sync.dma_start(b1_sb[:], b1[:])
            nc.sync.dma_start(bo_sb[:], bo[:])
            nc.sync.dma_start(ones_sb[:], ones[:])

            # state double buffers: [p] read at superstep s (s%2==p), other written
            h0_sb = [spool.tile([128, KH * B], FP, tag=f"h0T{i}", name=f"h0T{i}") for i in (0, 1)]
            h1_sb = [spool.tile([128, KH * B], FP, tag=f"h1T{i}", name=f"h1T{i}") for i in (0, 1)]
            c0_sb = spool.tile([HC, B], F32, tag="c0")
            c1_sb = spool.tile([HC, B], F32, tag="c1")
            nc.sync.dma_start(h0_sb[0][:].rearrange("p (k b) -> p k b", k=KH), h0T[:].rearrange("k p b -> p k b"))
            nc.sync.dma_start(c0_sb[:], c0T[:])
            nc.sync.dma_start(c1_sb[:], c0T[:])
            # my chunk of the initial state, staged for the s=0 gather (layer1
            # starts from the same (h0, c0) as layer0)
            hci_sb = spool.tile([HC, B], FP, tag="hci")
            nc.sync.dma_start(hci_sb[:], hci[:])
            warm = spool.tile([HC, B], F32, tag="warm")
            warm2 = spool.tile([HC, B], F32, tag="warm2")
            nc.vector.tensor_copy(warm[:], c0_sb[:])
            nc.vector.tensor_copy(warm2[:], c1_sb[:])
            nc.vector.tensor_add(warm[:], warm[:], warm2[:])

            h0new, h1new = hci_sb, hci_sb
            for s in range(t_steps + 1):
                p = s % 2
                q = 1 - p
                h0cur, h1cur = h0_sb[p], h1_sb[p]
                if s < t_steps:
                    xt = xpool.tile([D, B], FP, tag="xt")
                    nc.sync.dma_start(xt[:], xT[s])
                    g0 = [ppool.tile([128, B], F32, tag=f"g0_{m}", name=f"g0_{m}_{s}") for m in range(4)]
                    for m in range(4):
                        nc.tensor.matmul(
                            g0[m][:], b0_sb[:, m * 128:(m + 1) * 128], ones_sb[:],
                            start=True, stop=False)
                        nc.tensor.matmul(
                            g0[m][:], w0i_sb[:, m * 128:(m + 1) * 128], xt[:],
                            start=False, stop=False)
                        for k in range(KH):
                            nc.tensor.matmul(
                                g0[m][:],
                                w0h_sb[:, k * GC + m * 128: k * GC + (m + 1) * 128],
                                h0cur[:, k * B:(k + 1) * B],
                                start=False, stop=(k == KH - 1))
                    h0new = _act_block(nc, apool, g0, b0_sb, c0_sb, "l0")
                if s >= 1:
                    g1 = [ppool.tile([128, B], F32, tag=f"g1_{m}", name=f"g1_{m}_{s}") for m in range(4)]
                    for m in range(4):
                        nc.tensor.matmul(
                            g1[m][:], b1_sb[:, m * 128:(m + 1) * 128], ones_sb[:],
                            start=True, stop=False)
                        for k in range(KH):
                            nc.tensor.matmul(
                                g1[m][:],
                                w1i_sb[:, k * GC + m * 128: k * GC + (m + 1) * 128],
                                h0cur[:, k * B:(k + 1) * B],
                                start=False, stop=False)
                        for k in range(KH):
                            nc.tensor.matmul(
                                g1[m][:],
                                w1h_sb[:, k * GC + m * 128: k * GC + (m + 1) * 128],
                                h1cur[:, k * B:(k + 1) * B],
                                start=False, stop=(k == KH - 1))
                    h1new = _act_block(nc, apool, g1, b1_sb, c1_sb, "l1")

                # all-gather h0new (layer0 step s) and h1new (layer1 step s-1)
                cc_in = dpool.tile([2, HC, B], FP, tag="cc_in")
                cc_out = dpool.tile([NC, 2, HC, B], FP, tag="cc_out")
                nc.gpsimd.dma_start(cc_in[0], h0new[:])
                nc.gpsimd.dma_start(cc_in[1], h1new[:])
                nc.gpsimd.collective_compute(
                    "AllGather", mybir.AluOpType.bypass, replica_groups=RG,
                    ins=[cc_in.opt()], outs=[cc_out.opt()])
                nc.gpsimd.dma_start(
                    h0_sb[q][:].rearrange("p (k b) -> p k b", k=KH),
                    cc_out[:, 0].rearrange("k p b -> p k b"))
                nc.gpsimd.dma_start(
                    h1_sb[q][:].rearrange("p (k b) -> p k b", k=KH),
                    cc_out[:, 1].rearrange("k p b -> p k b"))

            # output projection: out^T[O, B] = W_out @ h1_T^T (+ b_out)
            pfin = (t_steps + 1) % 2
            po = ppool.tile([O, B], F32, tag="g0_0")
            nc.tensor.matmul(po[:], bo_sb[:], ones_sb[:], start=True, stop=False)
            for k in range(KH):
                nc.tensor.matmul(
                    po[:], wo_sb[:, k * O:(k + 1) * O],
                    h1_sb[pfin][:, k * B:(k + 1) * B],
                    start=False, stop=(k == KH - 1))
            out_sb = apool.tile([O, B], F32, tag="out")
            nc.scalar.copy(out_sb[:], po[:])
            nc.sync.dma_start(outT[:], out_sb[:])

    _split_excess_waits(nc)
    return nc


def _split_excess_waits(nc):
    """This walrus build embeds at most ONE sync wait per instruction (any
    type).  Move excess waits onto same-engine drains inserted immediately
    before the instruction, one wait per drain — engine queues execute in
    order, so semantics are unchanged."""
    for bb in nc.main_func.blocks:
        insts = list(bb.instructions)
        inserts = {}
        extras = []
        for pos, ins in enumerate(insts):
            si = ins.sync_info
            if si is None or not si.on_wait or len(si.on_wait) <= 1:
                continue
            waits = list(si.on_wait)
            keep, excess = waits[-1:], waits[:-1]
            carriers = []
            for w in excess:
                d = nc.engines[ins.engine].drain(fusable=False).ins
                d.sync_info = mybir.SyncInfo(on_wait=[w], on_update=[])
                carriers.append(d)
                extras.append(d)
            inserts[pos] = carriers
            si.on_wait = keep
            ins.sync_info = si
        if not inserts:
            continue
        extra_set = set(id(e) for e in extras)
        for blk in nc.main_func.blocks:
            blk.instructions = [i for i in blk.instructions
                                if id(i) not in extra_set]
        out = []
        for pos, ins in enumerate(insts):
            out.extend(inserts.get(pos, ()))
            out.append(ins)
        bb.instructions = out


def make_in_maps(x, h0, c0, W_ih0, W_hh0, b_ih0, b_hh0,
                 W_ih1, W_hh1, b_ih1, b_hh1, W_out, b_out, t_steps):
    xT = np.ascontiguousarray(
        np.transpose(x[:, :t_steps, :], (1, 2, 0))).astype(np.float16)
    h0T_full = np.ascontiguousarray(h0.T).astype(np.float16).reshape(KH, 128, B)
    c0T_full = np.ascontiguousarray(c0.T).astype(np.float32)
    wo_host = np.ascontiguousarray(W_out.T).astype(np.float16).reshape(KH, 128, O)
    bo_host = b_out.astype(np.float16).reshape(1, O)
    in_maps = []
    for j in range(NC):
        idx = np.concatenate(
            [np.arange(g * H + j * HC, g * H + (j + 1) * HC) for g in range(4)])
        w0i_j = np.ascontiguousarray(W_ih0[idx].T).astype(np.float16)
        w0h_j = np.ascontiguousarray(W_hh0[idx].T).astype(np.float16).reshape(KH, 128, GC)
        w1i_j = np.ascontiguousarray(W_ih1[idx].T).astype(np.float16).reshape(KH, 128, GC)
        w1h_j = np.ascontiguousarray(W_hh1[idx].T).astype(np.float16).reshape(KH, 128, GC)
        b0_j = (b_ih0 + b_hh0)[idx].reshape(1, GC).astype(np.float16)
        b1_j = (b_ih1 + b_hh1)[idx].reshape(1, GC).astype(np.float16)
        in_maps.append({
            "xT": xT, "w0i": w0i_j, "w0h": w0h_j, "w1i": w1i_j, "w1h": w1h_j,
            "wo": wo_host, "b0": b0_j, "b1": b1_j, "bo": bo_host,
            "ones": np.ones((1, B), np.float16),
            "h0T": h0T_full, "hci": h0T_full[j], "c0T": c0T_full[j * HC:(j + 1) * HC],
        })
    return in_maps


def run(t_steps, in_maps, trace=False):
    nc = build(t_steps)
    res = run_bass_kernel_spmd(nc, in_maps, list(range(NC)), trace=trace)
    return res


def kernel(**inputs):
    args = {k: np.asarray(v) for k, v in inputs.items()}
    in_maps = make_in_maps(
        args["x"], args["h0"], args["c0"],
        args["W_ih0"], args["W_hh0"], args["b_ih0"], args["b_hh0"],
        args["W_ih1"], args["W_hh1"], args["b_ih1"], args["b_hh1"],
        args["W_out"], args["b_out"], T)
    res = run(T, in_maps)
    outT = res.results[0]["outT"]
    return np.ascontiguousarray(outT.T).astype(np.float32)



# revision 2
# speedup vs baseline: 148.4066x; 148.4066x over previous
"""Two-layer LSTM (B=256, T=256, D=128, H=1024, O=128) on 8 trn2 NeuronCores.

Strategy: 8-way tensor-parallel over the 4H gate dimension.  Core j owns the
j-th 128-row slice of each gate block (i/f/g/o) of every weight matrix, i.e.
512 gate rows per layer, so it computes exactly the gates needed to update its
own 128-wide slice of h and c.  State is kept transposed (h^T [H, B]) so the
per-step matmuls are
    g^T[512, B] = W_chunk[512, K] @ h^T[K, B]
with the weight tiles stationary (lhsT) and h^T the moving operand (N=256).
Cell state c stays core-local in fp32; h chunks are all-gathered (fp16,
128KB/rank) once per superstep.  The two layers are pipelined: superstep s
computes layer0 step s and layer1 step s-1, so layer1's matmuls hide the
all-gather latency of layer0's recurrence.  A final AllGather + on-device
projection produces out^T = W_out @ h1_T; the host transposes it back.
"""

import numpy as np

import concourse.bass as bass
from concourse import bacc
import concourse.mybir as mybir
import concourse.tile as tile
from concourse.bass_utils import run_bass_kernel_spmd

B, T, D, H, O = 256, 256, 128, 1024, 128
NC = 8
HC = H // NC          # 128 h rows per core
GC = 4 * HC           # 512 gate rows per core
KH = H // 128         # 8 k-chunks over H
FP = mybir.dt.float16
F32 = mybir.dt.float32
AFT = mybir.ActivationFunctionType
RG = [list(range(NC))]


def _act_block(nc, apool, g, bb, c_sb, tag):
    """LSTM gate activations + cell update for one 128-slice.

    g: psum tiles [i, f, g, o] each [128, B] fp32.  bb: bias tile [128, 4]
    fp32 (folded into the scalar-engine activations, replacing the former
    per-step bias matmuls).  c_sb: fp32 cell tile [128, B], updated in place.
    Returns new h chunk [128, B] fp16."""
    it = apool.tile([HC, B], F32, tag=tag + "i")
    ft = apool.tile([HC, B], F32, tag=tag + "f")
    gt = apool.tile([HC, B], F32, tag=tag + "g")
    ot = apool.tile([HC, B], F32, tag=tag + "o")
    nc.scalar.activation(it[:], g[0][:], AFT.Sigmoid, bias=bb[:, 0:1])
    nc.scalar.activation(ft[:], g[1][:], AFT.Sigmoid, bias=bb[:, 1:2])
    nc.scalar.activation(gt[:], g[2][:], AFT.Tanh, bias=bb[:, 2:3])
    nc.scalar.activation(ot[:], g[3][:], AFT.Sigmoid, bias=bb[:, 3:4])
    fc = apool.tile([HC, B], F32, tag=tag + "fc")
    ig = apool.tile([HC, B], F32, tag=tag + "ig")
    nc.vector.tensor_mul(fc[:], ft[:], c_sb[:])
    nc.vector.tensor_mul(ig[:], it[:], gt[:])
    nc.vector.tensor_add(c_sb[:], fc[:], ig[:])
    tc_ = apool.tile([HC, B], F32, tag=tag + "tc")
    nc.scalar.activation(tc_[:], c_sb[:], AFT.Tanh)
    hnew = apool.tile([HC, B], FP, tag=tag + "h")
    nc.vector.tensor_mul(hnew[:], ot[:], tc_[:])
    return hnew


def build(t_steps):
    nc = bass.Bass(num_devices=NC)

    xT = nc.dram_tensor("xT", [t_steps, D, B], FP, kind="ExternalInput")
    w0i = nc.dram_tensor("w0i", [D, GC], FP, kind="ExternalInput")
    w0h = nc.dram_tensor("w0h", [KH, 128, GC], FP, kind="ExternalInput")
    w1i = nc.dram_tensor("w1i", [KH, 128, GC], FP, kind="ExternalInput")
    w1h = nc.dram_tensor("w1h", [KH, 128, GC], FP, kind="ExternalInput")
    wo = nc.dram_tensor("wo", [KH, 128, O], FP, kind="ExternalInput")
    b0 = nc.dram_tensor("b0", [HC, 4], F32, kind="ExternalInput")
    b1 = nc.dram_tensor("b1", [HC, 4], F32, kind="ExternalInput")
    bo = nc.dram_tensor("bo", [O, 1], F32, kind="ExternalInput")
    h0T = nc.dram_tensor("h0T", [KH, 128, B], FP, kind="ExternalInput")
    hci = nc.dram_tensor("hci", [HC, B], FP, kind="ExternalInput")
    c0T = nc.dram_tensor("c0T", [HC, B], F32, kind="ExternalInput")
    outT = nc.dram_tensor("outT", [O, B], F32, kind="ExternalOutput")

    with tile.TileContext(nc) as tc:
        with (
            tc.tile_pool(name="wpool", bufs=1) as wpool,
            tc.tile_pool(name="spool", bufs=1) as spool,
            tc.tile_pool(name="xpool", bufs=3) as xpool,
            tc.tile_pool(name="apool", bufs=2) as apool,
            tc.tile_pool(name="ppool", bufs=1, space="PSUM") as ppool,
            tc.tile_pool(name="dpool", bufs=2, space="DRAM") as dpool,
        ):
            w0i_sb = wpool.tile([D, GC], FP, tag="w0i")
            w0h_sb = wpool.tile([128, KH * GC], FP, tag="w0h")
            w1i_sb = wpool.tile([128, KH * GC], FP, tag="w1i")
            w1h_sb = wpool.tile([128, KH * GC], FP, tag="w1h")
            wo_sb = wpool.tile([128, KH * O], FP, tag="wo")
            b0_sb = wpool.tile([HC, 4], F32, tag="b0")
            b1_sb = wpool.tile([HC, 4], F32, tag="b1")
            bo_sb = wpool.tile([O, 1], F32, tag="bo")
            nc.sync.dma_start(w0i_sb[:], w0i[:])
            nc.sync.dma_start(w0h_sb[:].rearrange("p (k m) -> p k m", k=KH), w0h[:].rearrange("k p m -> p k m"))
            nc.sync.dma_start(w1i_sb[:].rearrange("p (k m) -> p k m", k=KH), w1i[:].rearrange("k p m -> p k m"))
            nc.sync.dma_start(w1h_sb[:].rearrange("p (k m) -> p k m", k=KH), w1h[:].rearrange("k p m -> p k m"))
            nc.sync.dma_start(wo_sb[:].rearrange("p (k m) -> p k m", k=KH), wo[:].rearrange("k p m -> p k m"))
            nc.sync.dma_start(b0_sb[:], b0[:])
            nc.sync.dma_start(b1_sb[:], b1[:])
            nc.sync.dma_start(bo_sb[:], bo[:])

            # state double buffers: [p] read at superstep s (s%2==p), other written
            h0_sb = [spool.tile([128, KH * B], FP, tag=f"h0T{i}", name=f"h0T{i}") for i in (0, 1)]
            h1_sb = [spool.tile([128, KH * B], FP, tag=f"h1T{i}", name=f"h1T{i}") for i in (0, 1)]
            c0_sb = spool.tile([HC, B], F32, tag="c0")
            c1_sb = spool.tile([HC, B], F32, tag="c1")
            nc.sync.dma_start(h0_sb[0][:].rearrange("p (k b) -> p k b", k=KH), h0T[:].rearrange("k p b -> p k b"))
            nc.sync.dma_start(c0_sb[:], c0T[:])
            nc.sync.dma_start(c1_sb[:], c0T[:])
            # my chunk of the initial state, staged for the s=0 gather (layer1
            # starts from the same (h0, c0) as layer0)
            hci_sb = spool.tile([HC, B], FP, tag="hci")
            nc.sync.dma_start(hci_sb[:], hci[:])
            warm = spool.tile([HC, B], F32, tag="warm")
            warm2 = spool.tile([HC, B], F32, tag="warm2")
            nc.vector.tensor_copy(warm[:], c0_sb[:])
            nc.vector.tensor_copy(warm2[:], c1_sb[:])
            nc.vector.tensor_add(warm[:], warm[:], warm2[:])

            h0new, h1new = hci_sb, hci_sb
            for s in range(t_steps + 1):
                p = s % 2
                q = 1 - p
                h0cur, h1cur = h0_sb[p], h1_sb[p]
                if s < t_steps:
                    xt = xpool.tile([D, B], FP, tag="xt")
                    nc.sync.dma_start(xt[:], xT[s])
                    g0 = [ppool.tile([128, B], F32, tag=f"g0_{m}", name=f"g0_{m}_{s}") for m in range(4)]
                    for m in range(4):
                        nc.tensor.matmul(
                            g0[m][:], w0i_sb[:, m * 128:(m + 1) * 128], xt[:],
                            start=True, stop=False)
                        for k in range(KH):
                            nc.tensor.matmul(
                                g0[m][:],
                                w0h_sb[:, k * GC + m * 128: k * GC + (m + 1) * 128],
                                h0cur[:, k * B:(k + 1) * B],
                                start=False, stop=(k == KH - 1))
                    h0new = _act_block(nc, apool, g0, b0_sb, c0_sb, "l0")
                    cc_in = dpool.tile([2, HC, B], FP, tag="cc_in")
                    nc.gpsimd.dma_start(cc_in[0], h0new[:])
                if s >= 1:
                    g1 = [ppool.tile([128, B], F32, tag=f"g1_{m}", name=f"g1_{m}_{s}") for m in range(4)]
                    for m in range(4):
                        for k in range(KH):
                            nc.tensor.matmul(
                                g1[m][:],
                                w1i_sb[:, k * GC + m * 128: k * GC + (m + 1) * 128],
                                h0cur[:, k * B:(k + 1) * B],
                                start=(k == 0), stop=False)
                        for k in range(KH):
                            nc.tensor.matmul(
                                g1[m][:],
                                w1h_sb[:, k * GC + m * 128: k * GC + (m + 1) * 128],
                                h1cur[:, k * B:(k + 1) * B],
                                start=False, stop=(k == KH - 1))
                    h1new = _act_block(nc, apool, g1, b1_sb, c1_sb, "l1")

                # all-gather h0new (layer0 step s) and h1new (layer1 step s-1).
                # cc_in[0] was staged right after layer0's act so that DMA
                # overlaps layer1's matmuls; at the final superstep (no L0)
                # stage it here from the previous step's tile.
                if s >= t_steps:
                    cc_in = dpool.tile([2, HC, B], FP, tag="cc_in")
                    nc.gpsimd.dma_start(cc_in[0], h0new[:])
                cc_out = dpool.tile([NC, 2, HC, B], FP, tag="cc_out")
                nc.gpsimd.dma_start(cc_in[1], h1new[:])
                nc.gpsimd.collective_compute(
                    "AllGather", mybir.AluOpType.bypass, replica_groups=RG,
                    ins=[cc_in.opt()], outs=[cc_out.opt()])
                # rearranges on separate hwdge queues so they run in parallel
                nc.sync.dma_start(
                    h0_sb[q][:].rearrange("p (k b) -> p k b", k=KH),
                    cc_out[:, 0].rearrange("k p b -> p k b"))
                nc.scalar.dma_start(
                    h1_sb[q][:].rearrange("p (k b) -> p k b", k=KH),
                    cc_out[:, 1].rearrange("k p b -> p k b"))

            # output projection: out^T[O, B] = W_out @ h1_T^T (+ b_out)
            pfin = (t_steps + 1) % 2
            po = ppool.tile([O, B], F32, tag="g0_0")
            for k in range(KH):
                nc.tensor.matmul(
                    po[:], wo_sb[:, k * O:(k + 1) * O],
                    h1_sb[pfin][:, k * B:(k + 1) * B],
                    start=(k == 0), stop=(k == KH - 1))
            out_sb = apool.tile([O, B], F32, tag="out")
            nc.scalar.activation(out_sb[:], po[:], AFT.Identity, bias=bo_sb[:, 0:1])
            nc.sync.dma_start(outT[:], out_sb[:])

    _split_excess_waits(nc)
    return nc


def _split_excess_waits(nc):
    """This walrus build embeds at most ONE sync wait per instruction (any
    type).  Move excess waits onto same-engine drains inserted immediately
    before the instruction, one wait per drain — engine queues execute in
    order, so semantics are unchanged."""
    for bb in nc.main_func.blocks:
        insts = list(bb.instructions)
        inserts = {}
        extras = []
        for pos, ins in enumerate(insts):
            si = ins.sync_info
            if si is None or not si.on_wait or len(si.on_wait) <= 1:
                continue
            waits = list(si.on_wait)
            keep, excess = waits[-1:], waits[:-1]
            carriers = []
            for w in excess:
                d = nc.engines[ins.engine].drain(fusable=False).ins
                d.sync_info = mybir.SyncInfo(on_wait=[w], on_update=[])
                carriers.append(d)
                extras.append(d)
            inserts[pos] = carriers
            si.on_wait = keep
            ins.sync_info = si
        if not inserts:
            continue
        extra_set = set(id(e) for e in extras)
        for blk in nc.main_func.blocks:
            blk.instructions = [i for i in blk.instructions
                                if id(i) not in extra_set]
        out = []
        for pos, ins in enumerate(insts):
            out.extend(inserts.get(pos, ()))
            out.append(ins)
        bb.instructions = out


def make_in_maps(x, h0, c0, W_ih0, W_hh0, b_ih0, b_hh0,
                 W_ih1, W_hh1, b_ih1, b_hh1, W_out, b_out, t_steps):
    xT = np.ascontiguousarray(
        np.transpose(x[:, :t_steps, :], (1, 2, 0))).astype(np.float16)
    h0T_full = np.ascontiguousarray(h0.T).astype(np.float16).reshape(KH, 128, B)
    c0T_full = np.ascontiguousarray(c0.T).astype(np.float32)
    wo_host = np.ascontiguousarray(W_out.T).astype(np.float16).reshape(KH, 128, O)
    bo_host = b_out.astype(np.float32).reshape(O, 1)
    in_maps = []
    for j in range(NC):
        idx = np.concatenate(
            [np.arange(g * H + j * HC, g * H + (j + 1) * HC) for g in range(4)])
        w0i_j = np.ascontiguousarray(W_ih0[idx].T).astype(np.float16)
        w0h_j = np.ascontiguousarray(W_hh0[idx].T).astype(np.float16).reshape(KH, 128, GC)
        w1i_j = np.ascontiguousarray(W_ih1[idx].T).astype(np.float16).reshape(KH, 128, GC)
        w1h_j = np.ascontiguousarray(W_hh1[idx].T).astype(np.float16).reshape(KH, 128, GC)
        b0_j = np.ascontiguousarray(
            (b_ih0 + b_hh0)[idx].reshape(4, HC).T).astype(np.float32)
        b1_j = np.ascontiguousarray(
            (b_ih1 + b_hh1)[idx].reshape(4, HC).T).astype(np.float32)
        in_maps.append({
            "xT": xT, "w0i": w0i_j, "w0h": w0h_j, "w1i": w1i_j, "w1h": w1h_j,
            "wo": wo_host, "b0": b0_j, "b1": b1_j, "bo": bo_host,
            "h0T": h0T_full, "hci": h0T_full[j], "c0T": c0T_full[j * HC:(j + 1) * HC],
        })
    return in_maps


def run(t_steps, in_maps, trace=False):
    nc = build(t_steps)
    res = run_bass_kernel_spmd(nc, in_maps, list(range(NC)), trace=trace)
    return res


def kernel(**inputs):
    args = {k: np.asarray(v) for k, v in inputs.items()}
    in_maps = make_in_maps(
        args["x"], args["h0"], args["c0"],
        args["W_ih0"], args["W_hh0"], args["b_ih0"], args["b_hh0"],
        args["W_ih1"], args["W_hh1"], args["b_ih1"], args["b_hh1"],
        args["W_out"], args["b_out"], T)
    res = run(T, in_maps)
    outT = res.results[0]["outT"]
    return np.ascontiguousarray(outT.T).astype(np.float32)

